# revision 10
# baseline (speedup 1.0000x reference)
"""Cross-attention kernel for TRN2, 8 NeuronCores.

Sharding: core (b, g) = batch b (4) x head-group g (2 groups of 4 heads).
Each core computes q/k/v projections for its 4 heads on its batch, full
T x (T+2) attention for those heads, and a partial output projection
(contribution of its 4 heads to out = attn @ Wo.T). Host sums the two
partials per batch and adds the constant (bo + Wo @ bv) term.

Math notes (vs reference):
  - 1/sqrt(Dh) folded into Wq/bq host-side.
  - tanh(g) folded into the advisory-token stream host-side
    (hpTs = hp * tanh(g), bkad = bk * tanh(g)).
  - softmax computed without max-subtraction (scores are O(5), exp is
    safe in fp32/bf16 range for this data distribution).
  - v-bias handled exactly on host: since rows of softmax sum to 1,
    its contribution to the output is the constant Wo @ bv.
  - all matmuls in bf16 with fp32 PSUM accumulation.

Schedule (v3): the scalar engine (ACT) is the attention bottleneck
(softmax exp is ~110us/core of ACT work at 128 lanes) and the start of
the kernel is input-DMA bound (~8.4MB/core, HBM shared with the paired
core). So the whole kernel is ONE pipeline of 16 attention steps
(head-major, 4 t-tiles each); all projection work is emitted as
"filler" PE matmul groups in slots between the score and AV matmuls of
each step:

  step (0,0): fillers = k[0] t1-3, v chunks (just-in-time for the AV
              chunk order), kad[0], vad, q[0] t1  (the k[0]t0/q[0]t0
              mini-prefix runs before the first step)
  (0,1..3):   q[0] t2-3 + q/k/kad[1]
  head 1,2:   q/k/kad[h+1]
  head 3:     o-proj of t-tile tti-1 (8 half-groups per step);
              o-proj of the last tile after the loop.

ACT does *only* exp (two-chunk [128,1024] PSUM reads halve its
~352-cycle per-instruction overhead); bias-adds and PSUM drains run on
the vector engine; the softmax 1/rowsum uses reciprocal_approx_fast +
gpsimd.partition_broadcast. Two filler groups per step are reserved to
sit between the tail-score and rowsum matmuls, hiding the ACT/DVE
latency of the step finale from the PE stream.

PSUM (16KB free space / partition, 2KB banks, exact fit):
  psS score pairs [128,2,512] f32 x2 bufs = 8KB
  psO AV accumulators [128,512] f32 x2    = 4KB
  psW filler/tail/rowsum slots x2         = 4KB
"""

import math
from collections import deque

import numpy as np
import ml_dtypes

import concourse.bass as bass
import concourse.mybir as mybir
import concourse.tile as tile
from concourse import bacc
from concourse.bass_utils import run_bass_kernel_spmd

BF16 = mybir.dt.bfloat16
F32 = mybir.dt.float32
AFT = mybir.ActivationFunctionType

P = 128
B, T, DIM = 4, 2048, 1024
NH, DH = 8, 128
HPG = 4              # heads per core
GD = HPG * DH        # 512 out-dims per core
KC = DIM // P        # 8 contraction chunks of the model dim
TT = 512             # t tile for attention
NT = T // TT         # 4 t tiles
NTC = T // P         # 16 t chunks of 128 (v layout, o-proj)
SFC = T // P         # 16 full s-chunks (key chunks of 128)
NSP = SFC // 2       # 8 score-chunk pairs per attention step
TPT = NTC // NT      # 4 t-chunks per t-tile

_CACHE = {}


def _build():
    nc = bacc.Bacc(
        "TRN2", target_bir_lowering=False, debug=False, enable_asserts=False
    )

    d = {}
    for name, shape, dt in [
        ("xT", [DIM, T], BF16),
        ("wqT", [DIM, GD], BF16),
        ("wkT", [DIM, GD], BF16),
        ("wvT", [DIM, GD], BF16),
        ("woT", [GD, DIM], BF16),
        ("bqv", [P, HPG], F32),
        ("bkv", [P, HPG], F32),
        ("bkad", [P, HPG], F32),
        ("hpT", [DIM, 2], BF16),
        ("hpTs", [DIM, 2], BF16),
    ]:
        d[name] = nc.dram_tensor(name, shape, dt, kind="ExternalInput").ap()
    out_ap = nc.dram_tensor("out", [T, DIM], BF16, kind="ExternalOutput").ap()

    with tile.TileContext(nc) as tc:
        with (
            tc.tile_pool(name="big", bufs=1) as big,
            tc.tile_pool(name="expp", bufs=3) as expp,
            tc.tile_pool(name="accp", bufs=2) as accp,
            tc.tile_pool(name="rcp", bufs=2) as rcp,
            tc.tile_pool(name="bcap", bufs=2) as bcap,
            tc.tile_pool(name="ostg", bufs=4) as ostg,
        ):
            # ---- persistent SBUF residents ----
            xt = big.tile([P, KC, T], BF16)
            wq = big.tile([P, KC, GD], BF16)
            wk = big.tile([P, KC, GD], BF16)
            wv = big.tile([P, KC, GD], BF16)
            wo = big.tile([P, HPG, DIM], BF16)
            bq_s = big.tile([P, HPG], F32)
            bk_s = big.tile([P, HPG], F32)
            bkad_s = big.tile([P, HPG], F32)
            hpt = big.tile([P, KC, 2], BF16)
            hpts = big.tile([P, KC, 2], BF16)
            qt = big.tile([P, HPG, T], BF16)
            kt = big.tile([P, HPG, T + 2], BF16)
            vsb = big.tile([P, NTC, GD], BF16)
            vad = big.tile([2, GD], BF16)
            ot = big.tile([P, HPG, T], BF16)
            ones_s = big.tile([P, 1], BF16)   # partition-sum lhsT

            # ---- input DMAs ----
            # sync (HWDGE): head-0 slice of wk + first t-tile of xT first
            # (unblocks k[0]t0 ASAP), then xT t-major, then rest of wk.
            # gpsimd (SWDGE): biases/hp, head-0 slice of wq, wv (needed by
            # the v fillers ~15us in), rest of wq, wo (needed ~170us in).
            xTd = d["xT"].rearrange("(c p) t -> p c t", p=P)
            wqd = d["wqT"].rearrange("(c p) f -> p c f", p=P)
            wkd = d["wkT"].rearrange("(c p) f -> p c f", p=P)
            wvd = d["wvT"].rearrange("(c p) f -> p c f", p=P)
            wod = d["woT"].rearrange("(c p) f -> p c f", p=P)
            t0 = slice(0, TT)
            for c in range(KC):
                nc.sync.dma_start(wk[:, c, 0:P], wkd[:, c, 0:P])
                nc.sync.dma_start(xt[:, c, t0], xTd[:, c, t0])
            for tti in range(1, NT):
                ts = slice(tti * TT, (tti + 1) * TT)
                for c in range(KC):
                    nc.sync.dma_start(xt[:, c, ts], xTd[:, c, ts])
            for c in range(KC):
                nc.sync.dma_start(wk[:, c, P:GD], wkd[:, c, P:GD])
            nc.gpsimd.dma_start(bq_s[:], d["bqv"][:])
            nc.gpsimd.dma_start(bk_s[:], d["bkv"][:])
            nc.gpsimd.dma_start(bkad_s[:], d["bkad"][:])
            nc.gpsimd.dma_start(hpt[:], d["hpT"].rearrange("(c p) t -> p c t", p=P))
            nc.gpsimd.dma_start(hpts[:], d["hpTs"].rearrange("(c p) t -> p c t", p=P))
            for c in range(KC):
                nc.gpsimd.dma_start(wq[:, c, 0:P], wqd[:, c, 0:P])
            for c in range(KC):
                nc.gpsimd.dma_start(wv[:, c, :], wvd[:, c, :])
            for c in range(KC):
                nc.gpsimd.dma_start(wq[:, c, P:GD], wqd[:, c, P:GD])
            for c in range(HPG):
                nc.gpsimd.dma_start(wo[:, c, :], wod[:, c, :])
            nc.vector.memset(ones_s[:], 1.0)

            with (
                tc.tile_pool(name="psS", bufs=2, space="PSUM") as psS,
                tc.tile_pool(name="psO", bufs=2, space="PSUM") as psO,
                tc.tile_pool(name="psW", bufs=2, space="PSUM") as psW,
            ):
                # ---- filler group emitters (one PE matmul group each) ----
                def wtile(name):
                    return psW.tile([P, TT], F32, tag="w", name=name)

                def g_qk(h, w, bias, dst, tti):
                    ts = slice(tti * TT, (tti + 1) * TT)
                    tag = "q" if dst is qt else "k"
                    ps = wtile(f"{tag}p_{h}_{tti}")
                    for c in range(KC):
                        nc.tensor.matmul(
                            ps[:], w[:, c, h * P : (h + 1) * P], xt[:, c, ts],
                            start=(c == 0), stop=(c == KC - 1),
                        )
                    nc.vector.tensor_scalar_add(
                        dst[:, h, ts], ps[:], bias[:, h : h + 1]
                    )

                def g_kad(h):
                    ps = wtile(f"kad_{h}")
                    for c in range(KC):
                        nc.tensor.matmul(
                            ps[:, 0:2], wk[:, c, h * P : (h + 1) * P],
                            hpts[:, c, :], start=(c == 0), stop=(c == KC - 1),
                        )
                    nc.vector.tensor_scalar_add(
                        kt[:, h, T : T + 2], ps[:, 0:2], bkad_s[:, h : h + 1]
                    )

                def g_v(tci):
                    ps = wtile(f"vp_{tci}")
                    for c in range(KC):
                        nc.tensor.matmul(
                            ps[:], xt[:, c, tci * P : (tci + 1) * P],
                            wv[:, c, :], start=(c == 0), stop=(c == KC - 1),
                        )
                    nc.vector.tensor_copy(vsb[:, tci, :], ps[:])

                def g_vad():
                    ps = wtile("vadp")
                    for c in range(KC):
                        nc.tensor.matmul(
                            ps[0:2, :], hpt[:, c, :], wv[:, c, :],
                            start=(c == 0), stop=(c == KC - 1),
                        )
                    nc.vector.tensor_copy(vad[:], ps[0:2, :])

                ostg_tiles = {}

                def g_oproj(tci, half):
                    ps = wtile(f"op_{tci}_{half}")
                    for c in range(HPG):
                        nc.tensor.matmul(
                            ps[:], ot[:, c, tci * P : (tci + 1) * P],
                            wo[:, c, half * 512 : (half + 1) * 512],
                            start=(c == 0), stop=(c == HPG - 1),
                        )
                    if half == 0:
                        ostg_tiles[tci] = ostg.tile(
                            [P, DIM], BF16, tag="ostage", name=f"o_{tci}"
                        )
                    stg = ostg_tiles[tci]
                    nc.vector.tensor_copy(
                        stg[:, half * 512 : (half + 1) * 512], ps[:]
                    )
                    if half == 1:
                        nc.sync.dma_start(
                            out_ap[tci * P : (tci + 1) * P, :], stg[:]
                        )

                # split o-proj for the last t-tile: heads 0-2 partials are
                # computed early (during step (3,0)); only head 3's single
                # matmul + the add remain after the final attention step.
                stage3 = {}

                def g_oproj3(tci, half):
                    ps = wtile(f"o3_{tci}_{half}")
                    for c in range(HPG - 1):
                        nc.tensor.matmul(
                            ps[:], ot[:, c, tci * P : (tci + 1) * P],
                            wo[:, c, half * 512 : (half + 1) * 512],
                            start=(c == 0), stop=(c == HPG - 2),
                        )
                    if half == 0:
                        stage3[tci] = ostg.tile(
                            [P, DIM], F32, tag="ostage3", name=f"s3_{tci}"
                        )
                    nc.vector.tensor_copy(
                        stage3[tci][:, half * 512 : (half + 1) * 512], ps[:]
                    )

                def g_oproj1(tci, half):
                    ps = wtile(f"o1_{tci}_{half}")
                    nc.tensor.matmul(
                        ps[:], ot[:, HPG - 1, tci * P : (tci + 1) * P],
                        wo[:, HPG - 1, half * 512 : (half + 1) * 512],
                        start=True, stop=True,
                    )
                    if half == 0:
                        ostg_tiles[tci] = ostg.tile(
                            [P, DIM], BF16, tag="ostage", name=f"o_{tci}"
                        )
                    stg = ostg_tiles[tci]
                    nc.vector.tensor_add(
                        stg[:, half * 512 : (half + 1) * 512],
                        stage3[tci][:, half * 512 : (half + 1) * 512], ps[:]
                    )
                    if half == 1:
                        nc.sync.dma_start(
                            out_ap[tci * P : (tci + 1) * P, :], stg[:]
                        )

                # ---- one attention step: head h, t-tile tti ----
                # slot_fillers: list of NSP lists of emitters (popped
                # between the slot's score and AV matmuls); post: emitters
                # placed between the tail-score and rowsum matmuls.
                def attn_step(h, tti, slot_fillers, post):
                    ts = slice(tti * TT, (tti + 1) * TT)
                    hs = slice(h * P, (h + 1) * P)
                    acc = accp.tile([P, TT], BF16, tag="acc",
                                    name=f"acc_{h}_{tti}")
                    ops = psO.tile([P, TT], F32, tag="av",
                                   name=f"av_{h}_{tti}")

                    def scores(k):
                        sp = psS.tile([P, 2, TT], F32, tag="sc",
                                      name=f"s_{h}_{tti}_{k}")
                        for j in range(2):
                            nc.tensor.matmul(
                                sp[:, j, :],
                                kt[:, h, (2 * k + j) * P : (2 * k + j + 1) * P],
                                qt[:, h, ts], start=True, stop=True,
                            )
                        return sp

                    prev = scores(0)
                    for k in range(NSP):
                        nxt = scores(k + 1) if k < NSP - 1 else None
                        for fn in slot_fillers[k]:
                            fn()
                        et = expp.tile([P, 2, TT], BF16, tag="exp",
                                       name=f"e_{h}_{tti}_{k}")
                        nc.scalar.activation(et[:], prev[:], AFT.Exp)
                        if k == 0:
                            nc.vector.tensor_copy(acc[:], et[:, 0, :])
                        else:
                            nc.vector.tensor_add(acc[:], acc[:], et[:, 0, :])
                        nc.vector.tensor_add(acc[:], acc[:], et[:, 1, :])
                        nc.tensor.matmul(ops[:], vsb[:, 2 * k, hs],
                                         et[:, 0, :], start=(k == 0),
                                         stop=False)
                        nc.tensor.matmul(ops[:], vsb[:, 2 * k + 1, hs],
                                         et[:, 1, :], start=False, stop=False)
                        prev = nxt
                    # advisory-token tail (2 extra keys); the reserved
                    # post fillers keep the PE busy while ACT/DVE finish
                    # the tail exp and the last acc adds.
                    tps = wtile(f"st_{h}_{tti}")
                    nc.tensor.matmul(tps[0:2, :], kt[:, h, T : T + 2],
                                     qt[:, h, ts], start=True, stop=True)
                    ett = expp.tile([2, TT], BF16, tag="exptail",
                                    name=f"et_{h}_{tti}")
                    nc.scalar.activation(ett[:], tps[0:2, :], AFT.Exp)
                    for fn in post:
                        fn()
                    nc.vector.tensor_add(acc[0:2, :], acc[0:2, :], ett[:])
                    nc.tensor.matmul(ops[:], vad[:, hs], ett[:],
                                     start=False, stop=True)
                    # denominators: partition-sum matmul, fast reciprocal,
                    # gpsimd partition-broadcast, normalize into oT.
                    smp = wtile(f"sm_{h}_{tti}")
                    nc.tensor.matmul(smp[0:1, :], ones_s[:], acc[:],
                                     start=True, stop=True)
                    rcf = rcp.tile([1, TT], F32, tag="recipf",
                                   name=f"rf_{h}_{tti}")
                    nc.vector.reciprocal_approx_fast(rcf[:], smp[0:1, :])
                    bsb = bcap.tile([P, TT], F32, tag="bcast",
                                    name=f"bs_{h}_{tti}")
                    nc.gpsimd.partition_broadcast(bsb[:], rcf[:])
                    nc.vector.tensor_mul(ot[:, h, ts], ops[:], bsb[:])

                # ---- mini-prefix: k[0]t0, q[0]t0 ----
                g_qk(0, wk, bk_s, kt, 0)
                g_qk(0, wq, bq_s, qt, 0)

                # ---- filler schedules ----
                def spread(items, nslots, start=0):
                    slots = [[] for _ in range(nslots)]
                    if not items:
                        return slots
                    span = nslots - start
                    for i, it in enumerate(items):
                        slots[start + i * span // len(items)].append(it)
                    return slots

                for h in range(HPG):
                    if h == 0:
                        # step (0,0): k[0] t1-3 and v chunks just-in-time
                        # (AV pair k consumes v chunks 2k,2k+1; scores
                        # pair k+1 needs k-tile (2k+3)//4 <= emitted).
                        s00 = [
                            [lambda: g_qk(0, wk, bk_s, kt, 1),
                             lambda: g_v(0), lambda: g_v(1)],
                            [lambda: g_qk(0, wk, bk_s, kt, 2),
                             lambda: g_v(2), lambda: g_v(3)],
                            [lambda: g_qk(0, wk, bk_s, kt, 3),
                             lambda: g_v(4), lambda: g_v(5)],
                            [lambda: g_v(6), lambda: g_v(7)],
                            [lambda: g_v(8), lambda: g_v(9)],
                            [lambda: g_v(10), lambda: g_v(11),
                             lambda: g_kad(0)],
                            [lambda: g_v(12), lambda: g_v(13), g_vad],
                            [lambda: g_v(14), lambda: g_v(15)],
                        ]
                        attn_step(0, 0, s00,
                                  [lambda: g_qk(0, wq, bq_s, qt, 1)])
                        rest = deque()
                        rest.append(lambda: g_qk(0, wq, bq_s, qt, 2))
                        rest.append(lambda: g_qk(0, wq, bq_s, qt, 3))
                        for tti in range(NT):
                            rest.append(
                                lambda t=tti: g_qk(1, wk, bk_s, kt, t))
                        rest.append(lambda: g_kad(1))
                        for tti in range(NT):
                            rest.append(
                                lambda t=tti: g_qk(1, wq, bq_s, qt, t))
                        # q[0]t2 must precede step (0,2); q[0]t3 step (0,3)
                        for tti in range(1, NT):
                            n = len(rest)
                            take = (n + (NT - 1 - tti)) // (NT - tti)
                            items = [rest.popleft() for _ in range(take)]
                            post = [items.pop()] if len(items) > 1 else []
                            attn_step(0, tti, spread(items, NSP), post)
                        continue
                    fillers = deque()
                    if h < HPG - 1:
                        nh = h + 1
                        for tti in range(NT):
                            fillers.append(
                                lambda t=tti, n=nh: g_qk(n, wk, bk_s, kt, t))
                        fillers.append(lambda n=nh: g_kad(n))
                        for tti in range(NT):
                            fillers.append(
                                lambda t=tti, n=nh: g_qk(n, wq, bq_s, qt, t))
                    for tti in range(NT):
                        if h == HPG - 1 and tti == 0:
                            for tci in range((NT - 1) * TPT, NT * TPT):
                                fillers.append(lambda c=tci: g_oproj3(c, 0))
                                fillers.append(lambda c=tci: g_oproj3(c, 1))
                        if h == HPG - 1 and tti > 0:
                            for tci in range((tti - 1) * TPT, tti * TPT):
                                fillers.append(lambda c=tci: g_oproj(c, 0))
                                fillers.append(lambda c=tci: g_oproj(c, 1))
                        if h == HPG - 1:
                            take = len(fillers)
                        else:
                            rem = NT - tti
                            take = (len(fillers) + rem - 1) // rem
                        items = [fillers.popleft() for _ in range(take)]
                        post = []
                        while items and len(post) < 2:
                            post.append(items.pop())
                        post.reverse()
                        # h3 fillers depend on the previous step's norm
                        # chain; start them at slot 2 so that latency
                        # hides behind the first score pairs.
                        attn_step(h, tti,
                                  spread(items, NSP,
                                         start=2 if h == HPG - 1 else 0),
                                  post)
                    while fillers:
                        fillers.popleft()()
                # final t-tile: head-3 contribution + staged heads 0-2
                for tci in range((NT - 1) * TPT, NT * TPT):
                    g_oproj1(tci, 0)
                    g_oproj1(tci, 1)

    nc.compile()
    return nc


def _get_nc():
    if "nc" not in _CACHE:
        _CACHE["nc"] = _build()
    return _CACHE["nc"]


def kernel(x, h, p, Wq, bq, Wk, bk, Wv, bv, Wo, bo, g, **_):
    x = np.asarray(x, np.float32)
    h = np.asarray(h, np.float32)
    p = np.asarray(p, np.float32)
    Wq = np.asarray(Wq, np.float32)
    bq = np.asarray(bq, np.float32)
    Wk = np.asarray(Wk, np.float32)
    bk = np.asarray(bk, np.float32)
    Wv = np.asarray(Wv, np.float32)
    bv = np.asarray(bv, np.float32)
    Wo = np.asarray(Wo, np.float32)
    bo = np.asarray(bo, np.float32)
    g = np.asarray(g, np.float32)

    nc = _get_nc()
    bf = ml_dtypes.bfloat16
    s = 1.0 / math.sqrt(DH)
    gt = float(np.tanh(g[0]))
    hp = np.concatenate([h, p], axis=1)  # [B, 2, DIM]

    per_group = []
    for gi in range(2):
        sl = slice(gi * GD, (gi + 1) * GD)
        per_group.append({
            "wqT": np.ascontiguousarray((Wq[sl] * s).T).astype(bf),
            "wkT": np.ascontiguousarray(Wk[sl].T).astype(bf),
            "wvT": np.ascontiguousarray(Wv[sl].T).astype(bf),
            "woT": np.ascontiguousarray(Wo[:, sl].T).astype(bf),
            "bqv": np.ascontiguousarray((bq[sl] * s).reshape(HPG, P).T,
                                        dtype=np.float32),
            "bkv": np.ascontiguousarray(bk[sl].reshape(HPG, P).T,
                                        dtype=np.float32),
            "bkad": np.ascontiguousarray((bk[sl] * gt).reshape(HPG, P).T,
                                         dtype=np.float32),
        })

    in_maps = []
    for b in range(B):
        xTb = np.ascontiguousarray(x[b].T).astype(bf)
        hpTb = np.ascontiguousarray(hp[b].T).astype(bf)
        hpTsb = np.ascontiguousarray((hp[b] * gt).T).astype(bf)
        for gi in range(2):
            m = dict(per_group[gi])
            m["xT"] = xTb
            m["hpT"] = hpTb
            m["hpTs"] = hpTsb
            in_maps.append(m)

    _CACHE["last_in_maps"] = in_maps
    res = run_bass_kernel_spmd(nc, in_maps, list(range(8)))
    outs = res.results

    const = (bo + Wo @ bv).astype(np.float32)
    out = np.empty((B, T, DIM), np.float32)
    for b in range(B):
        out[b] = (outs[2 * b]["out"].astype(np.float32)
                  + outs[2 * b + 1]["out"].astype(np.float32) + const)
    return out


# revision 11
# speedup vs baseline: 1.0003x; 1.0003x over previous
"""Cross-attention kernel for TRN2, 8 NeuronCores.

Sharding: core (b, g) = batch b (4) x head-group g (2 groups of 4 heads).
Each core computes q/k/v projections for its 4 heads on its batch, full
T x (T+2) attention for those heads, and a partial output projection
(contribution of its 4 heads to out = attn @ Wo.T). Host sums the two
partials per batch and adds the constant (bo + Wo @ bv) term.

Math notes (vs reference):
  - 1/sqrt(Dh) folded into Wq/bq host-side.
  - tanh(g) folded into the advisory-token stream host-side
    (hpTs = hp * tanh(g), bkad = bk * tanh(g)).
  - softmax computed without max-subtraction (scores are O(5), exp is
    safe in fp32/bf16 range for this data distribution).
  - v-bias handled exactly on host: since rows of softmax sum to 1,
    its contribution to the output is the constant Wo @ bv.
  - all matmuls in bf16 with fp32 PSUM accumulation.

Schedule (v3): the scalar engine (ACT) is the attention bottleneck
(softmax exp is ~110us/core of ACT work at 128 lanes) and the start of
the kernel is input-DMA bound (~8.4MB/core, HBM shared with the paired
core). So the whole kernel is ONE pipeline of 16 attention steps
(head-major, 4 t-tiles each); all projection work is emitted as
"filler" PE matmul groups in slots between the score and AV matmuls of
each step:

  step (0,0): fillers = k[0] t1-3, v chunks (just-in-time for the AV
              chunk order), kad[0], vad, q[0] t1  (the k[0]t0/q[0]t0
              mini-prefix runs before the first step)
  (0,1..3):   q[0] t2-3 + q/k/kad[1]
  head 1,2:   q/k/kad[h+1]
  head 3:     o-proj of t-tile tti-1 (8 half-groups per step);
              o-proj of the last tile after the loop.

ACT does *only* exp (two-chunk [128,1024] PSUM reads halve its
~352-cycle per-instruction overhead); bias-adds and PSUM drains run on
the vector engine; the softmax 1/rowsum uses reciprocal_approx_fast +
gpsimd.partition_broadcast. Two filler groups per step are reserved to
sit between the tail-score and rowsum matmuls, hiding the ACT/DVE
latency of the step finale from the PE stream.

PSUM (16KB free space / partition, 2KB banks, exact fit):
  psS score pairs [128,2,512] f32 x2 bufs = 8KB
  psO AV accumulators [128,512] f32 x2    = 4KB
  psW filler/tail/rowsum slots x2         = 4KB
"""

import math
from collections import deque

import numpy as np
import ml_dtypes

import concourse.bass as bass
import concourse.mybir as mybir
import concourse.tile as tile
from concourse import bacc
from concourse.bass_utils import run_bass_kernel_spmd

BF16 = mybir.dt.bfloat16
F32 = mybir.dt.float32
AFT = mybir.ActivationFunctionType

P = 128
B, T, DIM = 4, 2048, 1024
NH, DH = 8, 128
HPG = 4              # heads per core
GD = HPG * DH        # 512 out-dims per core
KC = DIM // P        # 8 contraction chunks of the model dim
TT = 512             # t tile for attention
NT = T // TT         # 4 t tiles
NTC = T // P         # 16 t chunks of 128 (v layout, o-proj)
SFC = T // P         # 16 full s-chunks (key chunks of 128)
NSP = SFC // 2       # 8 score-chunk pairs per attention step
TPT = NTC // NT      # 4 t-chunks per t-tile

_CACHE = {}


def _build():
    nc = bacc.Bacc(
        "TRN2", target_bir_lowering=False, debug=False, enable_asserts=False
    )

    d = {}
    for name, shape, dt in [
        ("xT", [DIM, T], BF16),
        ("wqT", [DIM, GD], BF16),
        ("wkT", [DIM, GD], BF16),
        ("wvT", [DIM, GD], BF16),
        ("woT", [GD, DIM], BF16),
        ("bqv", [P, HPG], F32),
        ("bkv", [P, HPG], F32),
        ("bkad", [P, HPG], F32),
        ("hpT", [DIM, 2], BF16),
        ("hpTs", [DIM, 2], BF16),
    ]:
        d[name] = nc.dram_tensor(name, shape, dt, kind="ExternalInput").ap()
    out_ap = nc.dram_tensor("out", [T, DIM], BF16, kind="ExternalOutput").ap()

    with tile.TileContext(nc) as tc:
        with (
            tc.tile_pool(name="big", bufs=1) as big,
            tc.tile_pool(name="expp", bufs=3) as expp,
            tc.tile_pool(name="accp", bufs=2) as accp,
            tc.tile_pool(name="rcp", bufs=2) as rcp,
            tc.tile_pool(name="bcap", bufs=2) as bcap,
            tc.tile_pool(name="ostg", bufs=4) as ostg,
        ):
            # ---- persistent SBUF residents ----
            xt = big.tile([P, KC, T], BF16)
            wq = big.tile([P, KC, GD], BF16)
            wk = big.tile([P, KC, GD], BF16)
            wv = big.tile([P, KC, GD], BF16)
            wo = big.tile([P, HPG, DIM], BF16)
            bq_s = big.tile([P, HPG], F32)
            bk_s = big.tile([P, HPG], F32)
            bkad_s = big.tile([P, HPG], F32)
            hpt = big.tile([P, KC, 2], BF16)
            hpts = big.tile([P, KC, 2], BF16)
            qt = big.tile([P, HPG, T], BF16)
            kt = big.tile([P, HPG, T + 2], BF16)
            vsb = big.tile([P, NTC, GD], BF16)
            vad = big.tile([2, GD], BF16)
            ot = big.tile([P, HPG, T], BF16)
            ones_s = big.tile([P, 1], BF16)   # partition-sum lhsT

            # ---- input DMAs ----
            # sync (HWDGE): head-0 slice of wk + first t-tile of xT first
            # (unblocks k[0]t0 ASAP), then xT t-major, then rest of wk.
            # gpsimd (SWDGE): biases/hp, head-0 slice of wq, wv (needed by
            # the v fillers ~15us in), rest of wq, wo (needed ~170us in).
            xTd = d["xT"].rearrange("(c p) t -> p c t", p=P)
            wqd = d["wqT"].rearrange("(c p) f -> p c f", p=P)
            wkd = d["wkT"].rearrange("(c p) f -> p c f", p=P)
            wvd = d["wvT"].rearrange("(c p) f -> p c f", p=P)
            wod = d["woT"].rearrange("(c p) f -> p c f", p=P)
            t0 = slice(0, TT)
            for c in range(KC):
                nc.sync.dma_start(wk[:, c, 0:P], wkd[:, c, 0:P])
                nc.sync.dma_start(xt[:, c, t0], xTd[:, c, t0])
            for tti in range(1, NT):
                ts = slice(tti * TT, (tti + 1) * TT)
                for c in range(KC):
                    nc.sync.dma_start(xt[:, c, ts], xTd[:, c, ts])
            for c in range(KC):
                nc.sync.dma_start(wk[:, c, P:GD], wkd[:, c, P:GD])
            nc.gpsimd.dma_start(bq_s[:], d["bqv"][:])
            nc.gpsimd.dma_start(bk_s[:], d["bkv"][:])
            nc.gpsimd.dma_start(bkad_s[:], d["bkad"][:])
            nc.gpsimd.dma_start(hpt[:], d["hpT"].rearrange("(c p) t -> p c t", p=P))
            nc.gpsimd.dma_start(hpts[:], d["hpTs"].rearrange("(c p) t -> p c t", p=P))
            for c in range(KC):
                nc.gpsimd.dma_start(wq[:, c, 0:P], wqd[:, c, 0:P])
            for c in range(KC):
                nc.gpsimd.dma_start(wv[:, c, :], wvd[:, c, :])
            for c in range(KC):
                nc.gpsimd.dma_start(wq[:, c, P:GD], wqd[:, c, P:GD])
            for c in range(HPG):
                nc.gpsimd.dma_start(wo[:, c, :], wod[:, c, :])
            nc.vector.memset(ones_s[:], 1.0)

            with (
                tc.tile_pool(name="psS", bufs=2, space="PSUM") as psS,
                tc.tile_pool(name="psO", bufs=2, space="PSUM") as psO,
                tc.tile_pool(name="psW", bufs=2, space="PSUM") as psW,
            ):
                # ---- filler group emitters (one PE matmul group each) ----
                def wtile(name):
                    return psW.tile([P, TT], F32, tag="w", name=name)

                def g_qk(h, w, bias, dst, tti):
                    ts = slice(tti * TT, (tti + 1) * TT)
                    tag = "q" if dst is qt else "k"
                    ps = wtile(f"{tag}p_{h}_{tti}")
                    for c in range(KC):
                        nc.tensor.matmul(
                            ps[:], w[:, c, h * P : (h + 1) * P], xt[:, c, ts],
                            start=(c == 0), stop=(c == KC - 1),
                        )
                    nc.vector.tensor_scalar_add(
                        dst[:, h, ts], ps[:], bias[:, h : h + 1]
                    )

                def g_kad(h):
                    ps = wtile(f"kad_{h}")
                    for c in range(KC):
                        nc.tensor.matmul(
                            ps[:, 0:2], wk[:, c, h * P : (h + 1) * P],
                            hpts[:, c, :], start=(c == 0), stop=(c == KC - 1),
                        )
                    nc.vector.tensor_scalar_add(
                        kt[:, h, T : T + 2], ps[:, 0:2], bkad_s[:, h : h + 1]
                    )

                def g_v(tci):
                    ps = wtile(f"vp_{tci}")
                    for c in range(KC):
                        nc.tensor.matmul(
                            ps[:], xt[:, c, tci * P : (tci + 1) * P],
                            wv[:, c, :], start=(c == 0), stop=(c == KC - 1),
                        )
                    nc.vector.tensor_copy(vsb[:, tci, :], ps[:])

                def g_vad():
                    ps = wtile("vadp")
                    for c in range(KC):
                        nc.tensor.matmul(
                            ps[0:2, :], hpt[:, c, :], wv[:, c, :],
                            start=(c == 0), stop=(c == KC - 1),
                        )
                    nc.vector.tensor_copy(vad[:], ps[0:2, :])

                ostg_tiles = {}

                def g_oproj(tci, half):
                    ps = wtile(f"op_{tci}_{half}")
                    for c in range(HPG):
                        nc.tensor.matmul(
                            ps[:], ot[:, c, tci * P : (tci + 1) * P],
                            wo[:, c, half * 512 : (half + 1) * 512],
                            start=(c == 0), stop=(c == HPG - 1),
                        )
                    if half == 0:
                        ostg_tiles[tci] = ostg.tile(
                            [P, DIM], BF16, tag="ostage", name=f"o_{tci}"
                        )
                    stg = ostg_tiles[tci]
                    nc.vector.tensor_copy(
                        stg[:, half * 512 : (half + 1) * 512], ps[:]
                    )
                    if half == 1:
                        nc.sync.dma_start(
                            out_ap[tci * P : (tci + 1) * P, :], stg[:]
                        )

                # split o-proj for the last t-tile: heads 0-2 partials are
                # computed early (during step (3,0)); only head 3's single
                # matmul + the add remain after the final attention step.
                stage3 = {}

                def g_oproj3(tci, half):
                    ps = wtile(f"o3_{tci}_{half}")
                    for c in range(HPG - 1):
                        nc.tensor.matmul(
                            ps[:], ot[:, c, tci * P : (tci + 1) * P],
                            wo[:, c, half * 512 : (half + 1) * 512],
                            start=(c == 0), stop=(c == HPG - 2),
                        )
                    if half == 0:
                        stage3[tci] = ostg.tile(
                            [P, DIM], F32, tag="ostage3", name=f"s3_{tci}"
                        )
                    nc.vector.tensor_copy(
                        stage3[tci][:, half * 512 : (half + 1) * 512], ps[:]
                    )

                def g_oproj1(tci, half):
                    ps = wtile(f"o1_{tci}_{half}")
                    nc.tensor.matmul(
                        ps[:], ot[:, HPG - 1, tci * P : (tci + 1) * P],
                        wo[:, HPG - 1, half * 512 : (half + 1) * 512],
                        start=True, stop=True,
                    )
                    if half == 0:
                        ostg_tiles[tci] = ostg.tile(
                            [P, DIM], BF16, tag="ostage", name=f"o_{tci}"
                        )
                    stg = ostg_tiles[tci]
                    nc.vector.tensor_add(
                        stg[:, half * 512 : (half + 1) * 512],
                        stage3[tci][:, half * 512 : (half + 1) * 512], ps[:]
                    )
                    if half == 1:
                        nc.sync.dma_start(
                            out_ap[tci * P : (tci + 1) * P, :], stg[:]
                        )

                # ---- one attention step: head h, t-tile tti ----
                # slot_fillers: list of NSP lists of emitters (popped
                # between the slot's score and AV matmuls); post: emitters
                # placed between the tail-score and rowsum matmuls.
                def attn_step(h, tti, slot_fillers, post):
                    ts = slice(tti * TT, (tti + 1) * TT)
                    hs = slice(h * P, (h + 1) * P)
                    acc = accp.tile([P, TT], BF16, tag="acc",
                                    name=f"acc_{h}_{tti}")
                    ops = psO.tile([P, TT], F32, tag="av",
                                   name=f"av_{h}_{tti}")

                    def scores(k):
                        sp = psS.tile([P, 2, TT], F32, tag="sc",
                                      name=f"s_{h}_{tti}_{k}")
                        for j in range(2):
                            nc.tensor.matmul(
                                sp[:, j, :],
                                kt[:, h, (2 * k + j) * P : (2 * k + j + 1) * P],
                                qt[:, h, ts], start=True, stop=True,
                            )
                        return sp

                    prev = scores(0)
                    # advisory-token tail hoisted to the step start: its
                    # exp is off the finale's critical path (the rowsum
                    # then only waits the last pair exp).
                    tps = wtile(f"st_{h}_{tti}")
                    nc.tensor.matmul(tps[0:2, :], kt[:, h, T : T + 2],
                                     qt[:, h, ts], start=True, stop=True)
                    ett = expp.tile([2, TT], BF16, tag="exptail",
                                    name=f"et_{h}_{tti}")
                    nc.scalar.activation(ett[:], tps[0:2, :], AFT.Exp)
                    for k in range(NSP):
                        nxt = scores(k + 1) if k < NSP - 1 else None
                        for fn in slot_fillers[k]:
                            fn()
                        et = expp.tile([P, 2, TT], BF16, tag="exp",
                                       name=f"e_{h}_{tti}_{k}")
                        nc.scalar.activation(et[:], prev[:], AFT.Exp)
                        if k == 0:
                            nc.vector.tensor_copy(acc[:], et[:, 0, :])
                        else:
                            nc.vector.tensor_add(acc[:], acc[:], et[:, 0, :])
                        nc.vector.tensor_add(acc[:], acc[:], et[:, 1, :])
                        if k == 0:
                            nc.vector.tensor_add(acc[0:2, :], acc[0:2, :],
                                                 ett[:])
                        nc.tensor.matmul(ops[:], vsb[:, 2 * k, hs],
                                         et[:, 0, :], start=(k == 0),
                                         stop=False)
                        nc.tensor.matmul(ops[:], vsb[:, 2 * k + 1, hs],
                                         et[:, 1, :], start=False, stop=False)
                        prev = nxt
                    # reserved post fillers keep the PE busy while DVE
                    # finishes the last acc adds before the rowsum.
                    for fn in post:
                        fn()
                    nc.tensor.matmul(ops[:], vad[:, hs], ett[:],
                                     start=False, stop=True)
                    # denominators: partition-sum matmul, fast reciprocal,
                    # gpsimd partition-broadcast, normalize into oT.
                    smp = wtile(f"sm_{h}_{tti}")
                    nc.tensor.matmul(smp[0:1, :], ones_s[:], acc[:],
                                     start=True, stop=True)
                    rcf = rcp.tile([1, TT], F32, tag="recipf",
                                   name=f"rf_{h}_{tti}")
                    nc.vector.reciprocal_approx_fast(rcf[:], smp[0:1, :])
                    bsb = bcap.tile([P, TT], F32, tag="bcast",
                                    name=f"bs_{h}_{tti}")
                    nc.gpsimd.partition_broadcast(bsb[:], rcf[:])
                    nc.vector.tensor_mul(ot[:, h, ts], ops[:], bsb[:])

                # ---- mini-prefix: k[0]t0, q[0]t0 ----
                g_qk(0, wk, bk_s, kt, 0)
                g_qk(0, wq, bq_s, qt, 0)

                # ---- filler schedules ----
                def spread(items, nslots, start=0):
                    slots = [[] for _ in range(nslots)]
                    if not items:
                        return slots
                    span = nslots - start
                    for i, it in enumerate(items):
                        slots[start + i * span // len(items)].append(it)
                    return slots

                for h in range(HPG):
                    if h == 0:
                        # step (0,0): k[0] t1-3 and v chunks just-in-time
                        # (AV pair k consumes v chunks 2k,2k+1; scores
                        # pair k+1 needs k-tile (2k+3)//4 <= emitted).
                        s00 = [
                            [lambda: g_qk(0, wk, bk_s, kt, 1),
                             lambda: g_v(0), lambda: g_v(1)],
                            [lambda: g_qk(0, wk, bk_s, kt, 2),
                             lambda: g_v(2), lambda: g_v(3)],
                            [lambda: g_qk(0, wk, bk_s, kt, 3),
                             lambda: g_v(4), lambda: g_v(5)],
                            [lambda: g_v(6), lambda: g_v(7)],
                            [lambda: g_v(8), lambda: g_v(9)],
                            [lambda: g_v(10), lambda: g_v(11),
                             lambda: g_kad(0)],
                            [lambda: g_v(12), lambda: g_v(13), g_vad],
                            [lambda: g_v(14), lambda: g_v(15)],
                        ]
                        attn_step(0, 0, s00,
                                  [lambda: g_qk(0, wq, bq_s, qt, 1)])
                        rest = deque()
                        rest.append(lambda: g_qk(0, wq, bq_s, qt, 2))
                        rest.append(lambda: g_qk(0, wq, bq_s, qt, 3))
                        for tti in range(NT):
                            rest.append(
                                lambda t=tti: g_qk(1, wk, bk_s, kt, t))
                        rest.append(lambda: g_kad(1))
                        for tti in range(NT):
                            rest.append(
                                lambda t=tti: g_qk(1, wq, bq_s, qt, t))
                        # q[0]t2 must precede step (0,2); q[0]t3 step (0,3)
                        for tti in range(1, NT):
                            n = len(rest)
                            take = (n + (NT - 1 - tti)) // (NT - tti)
                            items = [rest.popleft() for _ in range(take)]
                            post = [items.pop()] if len(items) > 1 else []
                            attn_step(0, tti, spread(items, NSP), post)
                        continue
                    fillers = deque()
                    if h < HPG - 1:
                        nh = h + 1
                        for tti in range(NT):
                            fillers.append(
                                lambda t=tti, n=nh: g_qk(n, wk, bk_s, kt, t))
                        fillers.append(lambda n=nh: g_kad(n))
                        for tti in range(NT):
                            fillers.append(
                                lambda t=tti, n=nh: g_qk(n, wq, bq_s, qt, t))
                    for tti in range(NT):
                        if h == HPG - 1 and tti == 0:
                            for tci in range((NT - 1) * TPT, NT * TPT):
                                fillers.append(lambda c=tci: g_oproj3(c, 0))
                                fillers.append(lambda c=tci: g_oproj3(c, 1))
                        if h == HPG - 1 and tti > 0:
                            for tci in range((tti - 1) * TPT, tti * TPT):
                                fillers.append(lambda c=tci: g_oproj(c, 0))
                                fillers.append(lambda c=tci: g_oproj(c, 1))
                        if h == HPG - 1:
                            take = len(fillers)
                        else:
                            rem = NT - tti
                            take = (len(fillers) + rem - 1) // rem
                        items = [fillers.popleft() for _ in range(take)]
                        post = []
                        while items and len(post) < 2:
                            post.append(items.pop())
                        post.reverse()
                        # h3 fillers depend on the previous step's norm
                        # chain; start them at slot 2 so that latency
                        # hides behind the first score pairs.
                        attn_step(h, tti,
                                  spread(items, NSP,
                                         start=2 if h == HPG - 1 else 0),
                                  post)
                    while fillers:
                        fillers.popleft()()
                # final t-tile: head-3 contribution + staged heads 0-2
                for tci in range((NT - 1) * TPT, NT * TPT):
                    g_oproj1(tci, 0)
                    g_oproj1(tci, 1)

    nc.compile()
    return nc


def _get_nc():
    if "nc" not in _CACHE:
        _CACHE["nc"] = _build()
    return _CACHE["nc"]


def kernel(x, h, p, Wq, bq, Wk, bk, Wv, bv, Wo, bo, g, **_):
    x = np.asarray(x, np.float32)
    h = np.asarray(h, np.float32)
    p = np.asarray(p, np.float32)
    Wq = np.asarray(Wq, np.float32)
    bq = np.asarray(bq, np.float32)
    Wk = np.asarray(Wk, np.float32)
    bk = np.asarray(bk, np.float32)
    Wv = np.asarray(Wv, np.float32)
    bv = np.asarray(bv, np.float32)
    Wo = np.asarray(Wo, np.float32)
    bo = np.asarray(bo, np.float32)
    g = np.asarray(g, np.float32)

    nc = _get_nc()
    bf = ml_dtypes.bfloat16
    s = 1.0 / math.sqrt(DH)
    gt = float(np.tanh(g[0]))
    hp = np.concatenate([h, p], axis=1)  # [B, 2, DIM]

    per_group = []
    for gi in range(2):
        sl = slice(gi * GD, (gi + 1) * GD)
        per_group.append({
            "wqT": np.ascontiguousarray((Wq[sl] * s).T).astype(bf),
            "wkT": np.ascontiguousarray(Wk[sl].T).astype(bf),
            "wvT": np.ascontiguousarray(Wv[sl].T).astype(bf),
            "woT": np.ascontiguousarray(Wo[:, sl].T).astype(bf),
            "bqv": np.ascontiguousarray((bq[sl] * s).reshape(HPG, P).T,
                                        dtype=np.float32),
            "bkv": np.ascontiguousarray(bk[sl].reshape(HPG, P).T,
                                        dtype=np.float32),
            "bkad": np.ascontiguousarray((bk[sl] * gt).reshape(HPG, P).T,
                                         dtype=np.float32),
        })

    in_maps = []
    for b in range(B):
        xTb = np.ascontiguousarray(x[b].T).astype(bf)
        hpTb = np.ascontiguousarray(hp[b].T).astype(bf)
        hpTsb = np.ascontiguousarray((hp[b] * gt).T).astype(bf)
        for gi in range(2):
            m = dict(per_group[gi])
            m["xT"] = xTb
            m["hpT"] = hpTb
            m["hpTs"] = hpTsb
            in_maps.append(m)

    _CACHE["last_in_maps"] = in_maps
    res = run_bass_kernel_spmd(nc, in_maps, list(range(8)))
    outs = res.results

    const = (bo + Wo @ bv).astype(np.float32)
    out = np.empty((B, T, DIM), np.float32)
    for b in range(B):
        out[b] = (outs[2 * b]["out"].astype(np.float32)
                  + outs[2 * b + 1]["out"].astype(np.float32) + const)
    return out
